# revision 50
# baseline (speedup 1.0000x reference)
"""CKConv Trainium2 kernel.

Math (derived from the reference):
  out[b,o,l] = sum_i sum_{d=0}^{l} g[o,i,d] * x[b,i,l-d] + conv_bias[o]
  g[o,i,d]   = k_full[o,i,2047-d],  k_full = w3 @ h2 + b3
  h2 = sin(30*(w2 @ h1 + b2)), h1 = sin(30*(w1 @ t + b1)), t = linspace(-1,1,L)

Mapping (per core, data-parallel over batch b):
  - Split l = 512p + n (p in 0..4), d = 128t + d'' (t in 0..16).  With
    s = 4p - t, the conv matmul rhs depends only on (i, s): all four p
    blocks stack into the PSUM partition dim -> 256 matmuls of
    [K=128, M=128, N<=512] fp16 accumulating into ONE psum bank
    P[32p+o, n] = out[o, 512p+n].  Causality kills s < -3, so
    s in [-3, 12] (16 values) x 16 i; for s < 0 the first -128s output
    columns read only left padding, so those windows shrink.
  - XS bank [128, 16*2560] fp16: XS[r, 2560i+c] = xpadh[i, c+r]
    (= x[i, c+r-638]), built by 16 overlapping-AP diagonal DMAs
    (one per i) straight from HBM -- all strides positive.
  - lhsT for (i, s): rows r of K16L hold g[o,i,128t+127-r].  The host
    permutes t4 (block-reversed within each 128-block) so the L3 psum
    rows come out reversed for free.  K16L col = 3584*r0+224*i+32*q+o
    (t' = t+12 = 4q+r0) makes each (i, s) lhsT a contiguous 128-col
    slice (matmul APs allow only one free dim); q <= 2 blocks are zeros
    (memset once, outside the timing loop) so p-slots with t < 0
    contribute exactly 0.
  - SIREN: L1 is an fp32 outer product on DVE/Pool (broadcast t via a
    stride-0 DMA); L2 a small PE matmul; fp32 range reduction
    (fused magic-number round) before ACT Sin.  Pool never touches
    PSUM (HW restriction) -- psum-reading steps go to DVE/ACT.
  - conv_bias enters as one K=1 matmul (cbm x ones) into the psum
    accumulator; epilogue casts PSUM->fp16 and DMAs 4 chunks on
    alternating queues (host casts back to fp32).
"""
import numpy as np

OMEGA0 = 30.0
CIN, COUT, HID = 16, 32, 32
B, L = 8, 2048
XPW = 2816         # padded x row width: 638 zeros | 2048 x | 130 zeros
XOFF = 638         # x starts at this col inside xpadh
XSW = 2560         # per-i XS row width
NT = 16            # d blocks (t)
NQ = 28            # t' blocks in K16L (t' = t + 12; first 12 are zeros)
PI = float(np.pi)
TWO_PI = float(2 * np.pi)
MAGIC = 12582912.0  # 1.5 * 2**23, fp32 round-to-nearest trick
INV_2PI = float(1.0 / (2 * np.pi))

_COMPILED = {}


def _build_host_inputs(w1, b1, w2, b2, w3, b3):
    """Small host-side layout prep of the SIREN weights (fp64 for exactness)."""
    w1 = np.asarray(w1, np.float64)  # [32, 1]
    b1 = np.asarray(b1, np.float64)  # [32]
    w2 = np.asarray(w2, np.float64)  # [32, 32]
    b2 = np.asarray(b2, np.float64)  # [32]
    w3 = np.asarray(w3, np.float64)  # [512, 32]
    b3 = np.asarray(b3, np.float64)  # [512]

    t = np.linspace(-1.0, 1.0, L)
    tr = -t  # reversed t
    # block-reverse within each 128-block: position fed at column 128t+m
    # is tr[128t + 127 - m], so L3 psum rows come out d''-reversed.
    trp = tr.reshape(NT, 128)[:, ::-1].reshape(L)
    t4p = trp.astype(np.float32)[None, :]                        # [1, L] f32
    # L1 runs on DVE as an outer product in fp32: w = trp*wb1[:,0] + wb1[:,1]
    wb1 = np.stack([np.tile(OMEGA0 * w1[:, 0] * INV_2PI, 4),
                    np.tile(OMEGA0 * b1 * INV_2PI, 4)], axis=1).astype(np.float32)

    a2 = np.tile((OMEGA0 * w2.T).astype(np.float16), (4, 1))     # [128, 32]
    # col 0: 30*b2 (DVE (ps+b)*inv path); col 1: 30*b2/2pi (ACT bias path)
    b2rep = np.stack([np.tile(OMEGA0 * b2, 4),
                      np.tile(OMEGA0 * b2 * INV_2PI, 4)],
                     axis=1).astype(np.float32)                  # [128, 2]

    # W3T[c, 32*i+o] = w3[16*o+i, c]; row 32 = b3[16*o+i]
    w3t = np.zeros((33, 512), np.float16)
    oi = np.arange(512)
    o, i = oi // CIN, oi % CIN
    f = 32 * i + o
    w3t[:32, f] = w3[oi, :].T.astype(np.float16)
    w3t[32, f] = b3[oi].astype(np.float16)
    return dict(t4p=t4p, wb1=wb1, a2=a2, b2rep=b2rep, w3t=w3t)


def _emit_prologue(nc, pool, ins, mybir, n_units=1):
    """One-time setup: shared XS bank tile plus, per pipeline unit, the
    constant zero blocks of K16L and the ones rows.

    The constants are input-independent; inside the For_i timing loop
    they persist across iterations.  With n_units=2 the loop body holds
    two back-to-back executions whose SIREN fronts overlap the other
    unit's conv tail (xs is shared; refills chase the reader per-range).
    """
    F16 = mybir.dt.float16
    xs = pool.tile([128, CIN * XSW], F16, name="xs")
    ones = pool.tile([1, 512], F16, name="ones")
    nc.vector.memset(ones[:, :], 1.0)
    units = []
    for u in range(n_units):
        k16l = pool.tile([128, NQ * 512], F16, name=f"k16l_{u}")
        k16r = k16l.rearrange("z (r0 i qo) -> z r0 i qo", i=CIN, qo=224)
        for r0 in range(4):
            nc.gpsimd.memset(k16r[:, r0, :, 0:96], 0.0)
        # H2 flat [33, 2048]: rows 0-31 features (per exec), row 32 ones
        h2 = pool.tile([33, L], F16, name=f"h2_{u}")
        nc.vector.memset(h2[32:33, :], 1.0)
        units.append((k16l, k16r, h2))
    return xs, ones, units


def _emit_smalls(nc, pool, ins, mybir, u):
    """Per-exec small-input DMAs (SP: trstack/wb1; ACT: the rest)."""
    import concourse.bass as bass
    F32 = mybir.dt.float32
    F16 = mybir.dt.float16
    # trstack[32a + f, n] = trp[512a + n] via a stride-0 broadcast DMA.
    trstack = pool.tile([128, 512], F32, name=f"trstack_{u}")
    nc.sync.dma_start(trstack[:, :],
                      bass.AP(ins["t4p"], 0, [[512, 4], [0, 32], [1, 512]]))
    wb1t = pool.tile([128, 2], F32, name=f"wb1t_{u}")
    nc.sync.dma_start(wb1t[:], ins["wb1"][:, :])
    a2t = pool.tile([128, 32], F16, name=f"a2t_{u}")
    nc.scalar.dma_start(a2t[:], ins["a2"][:, :])
    b2t = pool.tile([128, 2], F32, name=f"b2t_{u}")
    nc.scalar.dma_start(b2t[:], ins["b2rep"][:, :])
    w3tt = pool.tile([33, 512], F16, name=f"w3tt_{u}")
    nc.scalar.dma_start(w3tt[:], ins["w3t"][:, :])
    cbmt = pool.tile([1, 128], F16, name=f"cbmt_{u}")
    nc.scalar.dma_start(cbmt[:], ins["cbm"][:, :])
    return dict(trstack=trstack, wb1t=wb1t, a2t=a2t, b2t=b2t, w3tt=w3tt,
                cbmt=cbmt)


def _emit_xs(nc, ins, xs):
    """XS bank: 16 diagonal DMAs, one per i.
    XS[r, 2560*i + c] = xpadh[i, c + r]  (overlapping src AP).
    All on the SP queue: routing half over the ACT queue measured
    consistently slower (DMA triggers stall the ACT compute stream)."""
    import concourse.bass as bass
    for i in range(CIN):
        src = bass.AP(ins["xpadh"], i * XPW, [[1, 128], [1, XSW]])
        nc.sync.dma_start(xs[:, XSW * i:XSW * (i + 1)], src)


def _emit_front(nc, pool, pps, mybir, smalls, unit, u):
    """SIREN L1 (fp32 outer product on DVE/Pool) + L2 + h2 sins.

    d = w - round(w), w = trp*w1v + b1v (pre-divided by 2pi);
    round(w) = (w + M) - M in one fused tensor_scalar (fp32 rounds at +M).
    Pool cannot read PSUM, so psum-reading steps go to DVE / ACT.
    """
    k16l, k16r, h2 = unit
    F32 = mybir.dt.float32
    F16 = mybir.dt.float16
    AF = mybir.ActivationFunctionType
    OP = mybir.AluOpType

    def _round_sub(w_t, n_name, d_name):
        n_t = pool.tile([128, 512], F32, name=n_name)
        d_t = pool.tile([128, 512], F32, name=d_name)
        for eng, sl in ((nc.vector, slice(0, 256)), (nc.gpsimd, slice(256, 512))):
            eng.tensor_scalar(n_t[:, sl], w_t[:, sl], MAGIC, -MAGIC,
                              OP.add, OP.add)
            eng.tensor_tensor(d_t[:, sl], w_t[:, sl], n_t[:, sl], OP.subtract)
        return d_t

    w_t = pool.tile([128, 512], F32, name=f"w_t_{u}")
    for eng, sl in ((nc.vector, slice(0, 256)), (nc.gpsimd, slice(256, 512))):
        eng.tensor_scalar(w_t[:, sl], smalls["trstack"][:, sl],
                          smalls["wb1t"][:, 0:1], smalls["wb1t"][:, 1:2],
                          OP.mult, OP.add)
    d_t = _round_sub(w_t, f"n_t_{u}", f"d_t_{u}")
    h1 = pool.tile([128, 512], F16, name=f"h1_{u}")
    nc.scalar.activation(h1[:], d_t[:], AF.Sin, scale=TWO_PI)

    ps2 = pps.tile([128, 512], F32, name=f"ps2_{u}")
    for a in range(4):
        nc.tensor.matmul(ps2[32 * a:32 * a + 32, :],
                         smalls["a2t"][32 * a:32 * a + 32, :],
                         h1[32 * a:32 * a + 32, :],
                         start=True, stop=True,
                         tile_position=(32 * a, 32 * a))
    w2_t = pool.tile([128, 512], F32, name=f"w2_t_{u}")
    nc.vector.tensor_scalar(w2_t[:, 0:256], ps2[:, 0:256],
                            smalls["b2t"][:, 0:1], INV_2PI, OP.add, OP.mult)
    nc.scalar.activation(w2_t[:, 256:512], ps2[:, 256:512], AF.Identity,
                         bias=smalls["b2t"][:, 1:2], scale=INV_2PI)
    d2_t = _round_sub(w2_t, f"n2_t_{u}", f"d2_t_{u}")
    for a in range(4):
        nc.scalar.activation(h2[0:32, 512 * a:512 * a + 512],
                             d2_t[32 * a:32 * a + 32, :],
                             AF.Sin, scale=TWO_PI)


def _emit_l3_pair(nc, pps, mybir, smalls, unit, u, tp, ps3_bufs):
    """One L3 t-block pair -> K16L (pair shares q: clean 3-level dst AP).
    Pool cannot read PSUM: copies split DVE (10/16 i's) / ACT (6/16)."""
    k16l, k16r, h2 = unit
    F32 = mybir.dt.float32
    AF = mybir.ActivationFunctionType
    t = 2 * tp
    ps3 = pps.tile([128, 1024], F32, name=f"ps3_{(u * 8 + tp) % ps3_bufs}",
                   tag="ps3", bufs=ps3_bufs)
    for dt in range(2):
        nc.tensor.matmul(ps3[:, 512 * dt:512 * dt + 512],
                         h2[:, 128 * (t + dt):128 * (t + dt) + 128],
                         smalls["w3tt"][:, :],
                         start=True, stop=True)
    qt, rt = divmod(t + 12, 4)
    ps3h = ps3.rearrange("z (b io) -> z b io", b=2)
    nc.vector.tensor_copy(k16r[:, rt:rt + 2, 0:10, 32 * qt:32 * qt + 32],
                          ps3h[:, :, 0:320])
    nc.scalar.activation(k16r[:, rt:rt + 2, 10:16, 32 * qt:32 * qt + 32],
                         ps3h[:, :, 320:512], AF.Copy)


def _conv_order():
    order = []
    for i in range(CIN):
        ss = list(range(12, -4, -1))
        if i == CIN - 1:          # make the globally-last matmul full width
            ss = [-1, -2, -3] + list(range(12, -1, -1))
        order.extend((i, s) for s in ss)
    return order


def _emit_conv_part(nc, mybir, xs, ones, smalls, unit, P, order, lo, hi):
    """Conv matmuls [K=128, M=128, N<=512] for order[lo:hi].

    P[32p + o, n] = out[o, 512p + n]; for (i, s): rhs =
    XS[:, 2560i + 511 + 128s + n0 : +512], lhsT = K16L contiguous
    128 cols at (r0, i, q0) with q0, r0 = divmod(12 - s, 4).
    For s < 0 the columns n < -128s are entirely in the zero padding,
    so the window shrinks to n0 = -128s (saves 768 cycles per i).
    """
    k16l = unit[0]
    for idx in range(lo, hi):
        i, s = order[idx]
        q0, r0 = divmod(12 - s, 4)
        c0 = 3584 * r0 + 224 * i + 32 * q0
        lhsT = k16l[:, c0:c0 + 128]
        n0 = max(0, -128 * s)
        col = XSW * i + 511 + 128 * s + n0
        nc.tensor.matmul(P[:, n0:512], lhsT, xs[:, col:col + 512 - n0],
                         start=(idx == 0), stop=(idx == len(order) - 1))
        if idx == 0:
            # conv_bias as a rank-1 term: P[32p+o, :] += cbm[32p+o] * 1
            nc.tensor.matmul(P[:, :], smalls["cbmt"][:, :], ones[:, :],
                             start=False, stop=False)


def _emit_epilogue(nc, pool, ins, mybir, P, u):
    """Cast PSUM to fp16 + write out, chunk by chunk on both queues."""
    F16 = mybir.dt.float16
    AF = mybir.ActivationFunctionType
    out_sb = pool.tile([32, L], F16, name=f"out_sb_{u}")
    for p in range(4):
        if p % 2 == 0:
            nc.vector.tensor_copy(out_sb[:, 512 * p:512 * p + 512],
                                  P[32 * p:32 * p + 32, :])
        else:
            nc.scalar.activation(out_sb[:, 512 * p:512 * p + 512],
                                 P[32 * p:32 * p + 32, :], AF.Copy)
        deng = nc.sync if p % 2 == 0 else nc.scalar
        deng.dma_start(ins["out_res"][:, 512 * p:512 * p + 512],
                       out_sb[:, 512 * p:512 * p + 512])


def _emit_body(nc, tile, pool, pps, ins, mybir, xs, ones, unit, u=0,
               ps3_bufs=3):
    """Emit one full CKConv execution (the single-shot graded path)."""
    F32 = mybir.dt.float32
    smalls = _emit_smalls(nc, pool, ins, mybir, u)
    _emit_xs(nc, ins, xs)
    _emit_front(nc, pool, pps, mybir, smalls, unit, u)
    for tp in range(NT // 2):
        _emit_l3_pair(nc, pps, mybir, smalls, unit, u, tp, ps3_bufs)
    P = pps.tile([128, 512], F32, name=f"Pacc_{u}")
    order = _conv_order()
    _emit_conv_part(nc, mybir, xs, ones, smalls, unit, P, order, 0, len(order))
    _emit_epilogue(nc, pool, ins, mybir, P, u)


def _emit_chain(nc, tile, pool, pps, ins, mybir, xs, ones, units, nu):
    """nu pipelined executions per loop iteration, cycling two parity
    tile sets: unit u+1's SIREN front and L3 overlap unit u's conv (its
    L3 matmuls interleave into conv u's last four i-blocks on the PE
    queue), and unit u+1's XS refill chases conv u's reads before
    epilogue-u's DMAs.  Only unit 0's front and unit nu-1's epilogue
    stay exposed, amortized over nu executions.
    """
    F32 = mybir.dt.float32
    order = _conv_order()
    ng = len(order) // CIN        # matmuls per i-block (16)

    def unit(u):
        return units[u % 2]

    def conv_tail_with_next_front(u, P):
        # next unit's front under conv u; its L3 into conv u's tail
        _emit_front(nc, pool, pps, mybir, sm[u + 1], unit(u + 1),
                    (u + 1) % 2)
        for g in range(4):
            for tp in range(2 * g, 2 * g + 2):
                _emit_l3_pair(nc, pps, mybir, sm[u + 1], unit(u + 1),
                              (u + 1) % 2, tp, 2)
            _emit_conv_part(nc, mybir, xs, ones, sm[u], unit(u), P, order,
                            (12 + g) * ng, (13 + g) * ng)
        _emit_xs(nc, ins, xs)                      # refill for unit u+1

    sm = {0: _emit_smalls(nc, pool, ins, mybir, 0),
          1: _emit_smalls(nc, pool, ins, mybir, 1)}
    _emit_xs(nc, ins, xs)                          # fill for unit 0
    _emit_front(nc, pool, pps, mybir, sm[0], unit(0), 0)
    for tp in range(NT // 2):
        _emit_l3_pair(nc, pps, mybir, sm[0], unit(0), 0, tp, 2)

    P = {}
    for u in range(nu):
        P[u] = pps.tile([128, 512], F32, name=f"Pacc_{u % 2}")
        _emit_conv_part(nc, mybir, xs, ones, sm[u], unit(u), P[u], order,
                        0, 12 * ng)
        if u + 1 < nu:
            conv_tail_with_next_front(u, P[u])
            _emit_epilogue(nc, pool, ins, mybir, P[u], u % 2)
            if u + 2 < nu:
                sm[u + 2] = _emit_smalls(nc, pool, ins, mybir, (u + 2) % 2)
        else:
            _emit_conv_part(nc, mybir, xs, ones, sm[u], unit(u), P[u],
                            order, 12 * ng, len(order))
            _emit_epilogue(nc, pool, ins, mybir, P[u], u % 2)


def _gen(n_iters=1):
    import concourse.bass as bass
    import concourse.mybir as mybir
    import concourse.tile as tile
    from concourse import bacc

    F32 = mybir.dt.float32
    F16 = mybir.dt.float16

    nc = bacc.Bacc()
    ins = dict(
        xpadh=nc.dram_tensor("xpadh", [CIN, XPW], F16, kind="ExternalInput"),
        t4p=nc.dram_tensor("t4p", [1, L], F32, kind="ExternalInput"),
        wb1=nc.dram_tensor("wb1", [128, 2], F32, kind="ExternalInput"),
        a2=nc.dram_tensor("a2", [128, 32], F16, kind="ExternalInput"),
        b2rep=nc.dram_tensor("b2rep", [128, 2], F32, kind="ExternalInput"),
        w3t=nc.dram_tensor("w3t", [33, 512], F16, kind="ExternalInput"),
        cbm=nc.dram_tensor("cbm", [1, 128], F16, kind="ExternalInput"),
        out_res=nc.dram_tensor("out_res", [32, L], F16, kind="ExternalOutput"),
    )

    # n_iters == 1: single execution (the graded path).  n_iters > 1:
    # a For_i loop of n_iters iterations, each holding TWO back-to-back
    # executions (pipeline units) so unit 1's SIREN front overlaps unit
    # 0's conv tail; total executions per dispatch = 2 * n_iters.
    with tile.TileContext(nc) as tc:
        with tc.tile_pool(name="pool", bufs=1) as pool, \
             tc.tile_pool(name="pps", bufs=1, space="PSUM") as pps:
            if n_iters == 1:
                xs, ones, units = _emit_prologue(nc, pool, ins, mybir, 1)
                _emit_body(nc, tile, pool, pps, ins, mybir, xs, ones,
                           units[0], u=0, ps3_bufs=3)
            else:
                xs, ones, units = _emit_prologue(nc, pool, ins, mybir, 2)
                with tc.For_i(0, n_iters) as _:
                    _emit_chain(nc, tile, pool, pps, ins, mybir, xs, ones,
                                units, 4)

    nc.finalize()
    return nc


def _get_runner(n_iters=1):
    """Build (once) a cached jitted shard_map runner for the 8-core SPMD kernel."""
    key = f"runner_{n_iters}"
    if key in _COMPILED:
        return _COMPILED[key]

    import jax
    from jax.sharding import Mesh, PartitionSpec, NamedSharding
    from jax.experimental.shard_map import shard_map
    import concourse.mybir as mybir
    from concourse import bass2jax
    from concourse.bass2jax import _bass_exec_p, install_neuronx_cc_hook

    nc = _gen(n_iters)
    install_neuronx_cc_hook()

    partition_name = nc.partition_id_tensor.name if nc.partition_id_tensor else None
    in_names, out_names, out_avals, zero_outs = [], [], [], []
    for alloc in nc.m.functions[0].allocations:
        if not isinstance(alloc, mybir.MemoryLocationSet):
            continue
        name = alloc.memorylocations[0].name
        if alloc.kind == "ExternalInput":
            if name != partition_name:
                in_names.append(name)
        elif alloc.kind == "ExternalOutput":
            out_names.append(name)
            shape = tuple(alloc.tensor_shape)
            dtype = mybir.dt.np(alloc.dtype)
            out_avals.append(jax.core.ShapedArray(shape, dtype))
            zero_outs.append(np.zeros(shape, dtype))
    all_in_names = list(in_names) + list(out_names)
    if partition_name is not None:
        all_in_names.append(partition_name)

    def _body(*args):
        operands = list(args)
        if partition_name is not None:
            operands.append(bass2jax.partition_id_tensor())
        outs = _bass_exec_p.bind(
            *operands,
            out_avals=tuple(out_avals),
            in_names=tuple(all_in_names),
            out_names=tuple(out_names),
            lowering_input_output_aliases=(),
            sim_require_finite=True,
            sim_require_nnan=True,
            nc=nc,
        )
        return tuple(outs)

    devices = jax.devices()[:B]
    mesh = Mesh(np.asarray(devices, dtype=object), ("core",))
    n_args = len(in_names) + len(out_names)
    in_specs = (PartitionSpec("core"),) * n_args
    out_specs = (PartitionSpec("core"),) * len(out_names)
    sharded = jax.jit(
        shard_map(_body, mesh=mesh, in_specs=in_specs, out_specs=out_specs,
                  check_rep=False),
        keep_unused=True,
    )

    runner = dict(sharded=sharded, in_names=in_names, out_names=out_names,
                  out_avals=out_avals, zero_outs=zero_outs, mesh=mesh,
                  sharding=NamedSharding(mesh, PartitionSpec("core")))
    _COMPILED[key] = runner
    return runner


def _commit(in_maps, n_iters=1):
    """device_put the concatenated inputs (+ reusable zero outputs) once."""
    import jax
    r = _get_runner(n_iters)
    n_cores = len(in_maps)
    concat_in = [
        np.concatenate([np.asarray(m[name]) for m in in_maps], axis=0)
        for name in r["in_names"]
    ]
    concat_zeros = [np.zeros((n_cores * z.shape[0], *z.shape[1:]), z.dtype)
                    for z in r["zero_outs"]]
    return [jax.device_put(a, r["sharding"]) for a in (concat_in + concat_zeros)]


def _run_committed(args, n_iters=1):
    r = _get_runner(n_iters)
    return r["sharded"](*args)


def _run_spmd(in_maps):
    r = _get_runner(1)
    n_cores = len(in_maps)
    args = _commit(in_maps, 1)
    out_arrs = [np.asarray(a) for a in r["sharded"](*args)]
    return [
        {name: out_arrs[i].reshape(n_cores, *r["out_avals"][i].shape)[c]
         for i, name in enumerate(r["out_names"])}
        for c in range(n_cores)
    ]


def _make_in_maps(x, conv_bias, host):
    cb = np.asarray(conv_bias, np.float64)
    cbm = np.tile(cb, 4).astype(np.float16).reshape(1, 128)
    in_maps = []
    for b in range(B):
        xpadh = np.zeros((CIN, XPW), np.float16)
        xpadh[:, XOFF:XOFF + L] = x[b].astype(np.float16)
        in_maps.append(dict(xpadh=xpadh, cbm=cbm, **host))
    return in_maps


def _postprocess(results):
    out = np.zeros((B, COUT, L), np.float32)
    for b in range(B):
        out[b] = results[b]["out_res"].astype(np.float32)
    return out


def kernel(x, w1, b1, w2, b2, w3, b3, conv_bias):
    x = np.asarray(x)
    host = _build_host_inputs(w1, b1, w2, b2, w3, b3)
    in_maps = _make_in_maps(x, conv_bias, host)
    results = _run_spmd(in_maps)
    return _postprocess(results)
